# revision 20
# baseline (speedup 1.0000x reference)
"""Trainium2 Bass kernel for the bidirectional GRU-ODE (nn_CODEBiGRU).

Strategy (8-way TP, latency-minimized: per-instruction overhead dominates on
this setup, so the design minimizes serial instruction count):
  - State h is partition-tiled [p, ch, jt] (element 128*jt+p), replicated on
    every core, f32.
  - f-eval per RK4 stage: u = W1@hstage+b1 via one DVE mul (per-partition
    partial products, bias folded as an extra contraction slot), one DVE
    reduce, one gpsimd partition_all_reduce (result replicated across
    partitions, so tanh output is directly mv2's contraction input).
    k-partial via W2 column-shard: one DVE mul + one reduce; partials summed
    across cores by a single 32KB ncfw AllReduce per stage.
  - RK4 coefficients folded into small const tensors; ~4 DVE update ops/stage.
  - GRU + h2o phase at the end reuses the same pattern (2 AllGathers).
"""
import sys
import numpy as np

sys.path.insert(0, "/opt/trn_rl_repo")

import ml_dtypes  # noqa: E402
import concourse.bass as bass  # noqa: E402
import concourse.tile as tile  # noqa: E402
import concourse.bass_isa as bass_isa  # noqa: E402
from concourse import bacc, mybir, bass_utils  # noqa: E402

NCORES = 8
NH = 4096
R = NH // NCORES        # 512 rows per core
KT = NH // 128          # 32 k-tiles
NSTEP = 15
F32 = mybir.dt.float32
BF16 = mybir.dt.bfloat16
AF = mybir.ActivationFunctionType
ALU = mybir.AluOpType
AX = mybir.AxisListType
RED = bass_isa.ReduceOp
GROUP = [list(range(NCORES))]
BF = ml_dtypes.bfloat16


def _bc1(ap, shape):
    """[p, a, b] -> [p, N, a, b] broadcast (insert axis after partition)."""
    a = ap.shape[1]
    return ap.rearrange("p (o a) b -> p o a b", o=1, a=a).broadcast_to(shape)


def _bc2(ap, shape):
    """[p, ch, x] -> [p, ch, N, x] broadcast (insert axis 2)."""
    x = ap.shape[2]
    return ap.rearrange("p ch (o x) -> p ch o x", o=1, x=x).broadcast_to(shape)


def _build(niters=1):
    nc = bacc.Bacc("TRN2", target_bir_lowering=False, debug=False,
                   num_devices=NCORES)

    # ---- kernel I/O (per-core prepped on host) ----
    w1p_d = nc.dram_tensor("w1p", [128, R * 33], BF16, kind="ExternalInput")
    w2f_d = nc.dram_tensor("w2f", [128, KT * (R + 1)], BF16, kind="ExternalInput")
    wgx_d = nc.dram_tensor("wgx", [128, R * 33], BF16, kind="ExternalInput")
    wgh_d = nc.dram_tensor("wgh", [128, R * 32], BF16, kind="ExternalInput")
    wo_d = nc.dram_tensor("wo", [128, R * 65], BF16, kind="ExternalInput")
    xt_d = nc.dram_tensor("xt", [128, 66], BF16, kind="ExternalInput")
    h0bf_d = nc.dram_tensor("h0bf", [128, 66], BF16, kind="ExternalInput")
    h0f_d = nc.dram_tensor("h0f", [128, 64], F32, kind="ExternalInput")
    cw_d = nc.dram_tensor("cw", [128, NSTEP * 16], F32, kind="ExternalInput")

    o_slice = nc.dram_tensor("o_slice", [R], F32, kind="ExternalOutput")
    hf_out = nc.dram_tensor("hf_out", [NH], BF16, kind="ExternalOutput")
    hb_out = nc.dram_tensor("hb_out", [NH], BF16, kind="ExternalOutput")

    with tile.TileContext(nc) as tc:
        with tc.tile_pool(name="persist", bufs=1) as pp, \
             tc.tile_pool(name="dram", bufs=2, space="DRAM") as dram:

            h_t = pp.tile([128, 2, KT], F32, tag="h_t")
            xt = pp.tile([128, 2, 33], BF16, tag="xt")
            cw = pp.tile([128, NSTEP, 4, 2, 2], F32, tag="cw")
            hcat = pp.tile([128, 65], BF16, tag="hcat")

            arin = dram.tile([128, 64], F32, tag="arin", name="arin")
            arout = dram.tile([128, 64], F32, tag="arout", name="arout")
            agin = dram.tile([1, 2 * R], F32, tag="agin", name="agin")
            agout = dram.tile([NCORES, 2 * R], F32, tag="agout", name="agout")
            ag2in = dram.tile([1, 2 * R], F32, tag="ag2in", name="ag2in")
            ag2out = dram.tile([NCORES, 2 * R], F32, tag="ag2out", name="ag2out")

            for _it in range(niters):
                nc.sync.dma_start(xt[:].rearrange("p a b -> p (a b)"), xt_d[:])
                nc.sync.dma_start(h_t[:].rearrange("p a b -> p (a b)"),
                                  h0f_d[:])
                nc.sync.dma_start(
                    cw[:].rearrange("p a b c d -> p (a b c d)"), cw_d[:])

                # ================= ODE phase =================
                with tc.tile_pool(name="ode", bufs=1) as op_:
                    w1p = op_.tile([128, R, 33], BF16, tag="w1p")
                    w2f = op_.tile([128, KT, R + 1], BF16, tag="w2f")
                    scr = op_.tile([128, 2 * R * 33], BF16, tag="scr")
                    s1 = op_.tile([128, 2, R], F32, tag="s1")
                    urep = op_.tile([128, 2, R], F32, tag="urep")
                    t_rep = op_.tile([128, 2, R + 1], BF16, tag="t_rep")
                    hst = op_.tile([128, 2, 33], BF16, tag="hst")
                    hacc = op_.tile([128, 2, KT], F32, tag="hacc")
                    kf = op_.tile([128, 2, KT], F32, tag="kf")
                    kpart = op_.tile([128, 2, KT], F32, tag="kpart")
                    tmpv = op_.tile([128, 2, KT], F32, tag="tmpv")

                    nc.sync.dma_start(
                        w1p[:].rearrange("p a b -> p (a b)"), w1p_d[:])
                    nc.sync.dma_start(
                        w2f[:].rearrange("p a b -> p (a b)"), w2f_d[:])
                    nc.sync.dma_start(
                        hst[:].rearrange("p a b -> p (a b)"), h0bf_d[:])
                    nc.gpsimd.memset(t_rep[:, :, R:R + 1], 1.0)

                    A = scr[:].rearrange("p (ch j t) -> p ch j t", ch=2, j=R)
                    Bv = scr[:, :2 * KT * (R + 1)].rearrange(
                        "p (ch jt j) -> p ch jt j", ch=2, jt=KT)
                    w1b = _bc1(w1p[:], [128, 2, R, 33])
                    w2b = _bc1(w2f[:], [128, 2, KT, R + 1])

                    for s in range(NSTEP):
                        for q in range(4):
                            # u = W1 @ hstage + b1 (partials -> all partitions)
                            nc.vector.tensor_mul(
                                A, w1b, _bc2(hst[:], [128, 2, R, 33]))
                            nc.vector.reduce_sum(s1[:], A, axis=AX.X)
                            nc.gpsimd.partition_all_reduce(
                                urep[:].rearrange("p a b -> p (a b)"),
                                s1[:].rearrange("p a b -> p (a b)"),
                                128, RED.add)
                            nc.scalar.activation(t_rep[:, :, 0:R], urep[:],
                                                 AF.Tanh)
                            # k_partial = W2c @ t + b2/8 (tiled layout)
                            nc.vector.tensor_mul(
                                Bv, w2b, _bc2(t_rep[:], [128, 2, KT, R + 1]))
                            nc.vector.reduce_sum(kpart[:], Bv, axis=AX.X)
                            # AllReduce k across cores
                            nc.sync.dma_start(
                                arin[:], kpart[:].rearrange("p a b -> p (a b)"))
                            nc.gpsimd.collective_compute(
                                "AllReduce", ALU.add, replica_groups=GROUP,
                                ins=[arin.opt()], outs=[arout.opt()])
                            nc.sync.dma_start(
                                kf[:].rearrange("p a b -> p (a b)"), arout[:])
                            # RK4 bookkeeping
                            wqb = cw[:, s, q, 1, :].rearrange(
                                "p (ch o) -> p ch o", o=1
                            ).broadcast_to([128, 2, KT])
                            if q < 3:
                                cqb = cw[:, s, q, 0, :].rearrange(
                                    "p (ch o) -> p ch o", o=1
                                ).broadcast_to([128, 2, KT])
                                nc.vector.tensor_mul(tmpv[:], kf[:], cqb)
                                nc.vector.tensor_add(hst[:, :, 0:KT], h_t[:],
                                                     tmpv[:])
                                nc.vector.tensor_mul(tmpv[:], kf[:], wqb)
                                if q == 0:
                                    nc.vector.tensor_add(hacc[:], h_t[:],
                                                         tmpv[:])
                                else:
                                    nc.vector.tensor_add(hacc[:], hacc[:],
                                                         tmpv[:])
                            else:
                                nc.vector.tensor_mul(tmpv[:], kf[:], wqb)
                                nc.vector.tensor_add(h_t[:], hacc[:], tmpv[:])
                                if s < NSTEP - 1:
                                    nc.vector.tensor_copy(hst[:, :, 0:KT],
                                                          h_t[:])

                # ================= GRU + output phase =================
                with tc.tile_pool(name="gru", bufs=1) as gp:
                    wgx = gp.tile([128, R, 33], BF16, tag="wgx")
                    wgh = gp.tile([128, R, 32], BF16, tag="wgh")
                    scrg = gp.tile([128, 2 * R * 33], BF16, tag="scrg")
                    sx = gp.tile([128, 2, R], F32, tag="sx")
                    sh = gp.tile([128, 2, R], F32, tag="sh")
                    ug = gp.tile([128, 2, R], F32, tag="ug")
                    gs = gp.tile([128, R, 2], F32, tag="gs")
                    hh1 = gp.tile([1, R, 2], F32, tag="hh1")
                    vh = gp.tile([128, 2, KT], BF16, tag="vh")
                    gful = gp.tile([128, KT, 2], F32, tag="gful")
                    hhful = gp.tile([128, KT, 2], F32, tag="hhful")
                    tmpg = gp.tile([128, 2, KT], F32, tag="tmpg")

                    nc.sync.dma_start(
                        wgx[:].rearrange("p a b -> p (a b)"), wgx_d[:])
                    nc.sync.dma_start(
                        wgh[:].rearrange("p a b -> p (a b)"), wgh_d[:])

                    Ax = scrg[:].rearrange("p (ch j t) -> p ch j t", ch=2, j=R)
                    Ah = scrg[:, :2 * R * 32].rearrange(
                        "p (ch j t) -> p ch j t", ch=2, j=R)
                    wgxb = _bc1(wgx[:], [128, 2, R, 33])
                    wghb = _bc1(wgh[:], [128, 2, R, 32])

                    # ux = i2h_x @ x + bg  (partial, reused by both matvecs)
                    nc.vector.tensor_mul(Ax, wgxb, _bc2(xt[:], [128, 2, R, 33]))
                    nc.vector.reduce_sum(sx[:], Ax, axis=AX.X)
                    # uh = i2h_h @ h
                    nc.vector.tensor_copy(vh[:], h_t[:])
                    nc.vector.tensor_mul(Ah, wghb, _bc2(vh[:], [128, 2, R, 32]))
                    nc.vector.reduce_sum(sh[:], Ah, axis=AX.X)
                    nc.vector.tensor_add(sh[:], sx[:], sh[:])
                    nc.gpsimd.partition_all_reduce(
                        ug[:].rearrange("p a b -> p (a b)"),
                        sh[:].rearrange("p a b -> p (a b)"), 128, RED.add)
                    nc.scalar.activation(
                        gs[:].rearrange("p r ch -> p ch r"), ug[:],
                        AF.Sigmoid)
                    # AllGather g (block [r, ch]: flat idx 64p+2jt+ch)
                    nc.sync.dma_start(
                        agin[:], gs[0:1, :, :].rearrange("o r ch -> o (r ch)"))
                    nc.gpsimd.collective_compute(
                        "AllGather", ALU.bypass, replica_groups=GROUP,
                        ins=[agin.opt()], outs=[agout.opt()])
                    nc.sync.dma_start(
                        gful[:],
                        agout[:].rearrange("a rc -> (a rc)").rearrange(
                            "(p jt ch) -> p jt ch", p=128, ch=2))
                    # h_hat = tanh(ux + i2h_h @ (g*h))
                    gfv = gful[:].rearrange("p jt ch -> p ch jt")
                    hhv = hhful[:].rearrange("p jt ch -> p ch jt")
                    nc.vector.tensor_mul(vh[:], gfv, h_t[:])
                    nc.vector.tensor_mul(Ah, wghb, _bc2(vh[:], [128, 2, R, 32]))
                    nc.vector.reduce_sum(sh[:], Ah, axis=AX.X)
                    nc.vector.tensor_add(sh[:], sx[:], sh[:])
                    nc.gpsimd.partition_all_reduce(
                        ug[:].rearrange("p a b -> p (a b)"),
                        sh[:].rearrange("p a b -> p (a b)"), 128, RED.add)
                    nc.scalar.activation(
                        hh1[:].rearrange("o r ch -> o ch r"),
                        ug[0:1, :, :], AF.Tanh)
                    # AllGather h_hat
                    nc.sync.dma_start(
                        ag2in[:], hh1[:].rearrange("o r ch -> o (r ch)"))
                    nc.gpsimd.collective_compute(
                        "AllGather", ALU.bypass, replica_groups=GROUP,
                        ins=[ag2in.opt()], outs=[ag2out.opt()])
                    nc.sync.dma_start(
                        hhful[:],
                        ag2out[:].rearrange("a rc -> (a rc)").rearrange(
                            "(p jt ch) -> p jt ch", p=128, ch=2))
                    # h_new = hh + g*(h - hh)  -> hcat (bf16)
                    nc.vector.tensor_sub(tmpg[:], h_t[:], hhv)
                    nc.vector.tensor_mul(tmpg[:], gfv, tmpg[:])
                    nc.gpsimd.memset(hcat[:, 64:65], 1.0)
                    nc.vector.tensor_add(
                        hcat[:, 0:64].rearrange("p (ch jt) -> p ch jt", ch=2),
                        hhv, tmpg[:])
                    nc.sync.dma_start(
                        hf_out[:].rearrange("(p jt) -> p jt", p=128),
                        hcat[:, 0:KT])
                    nc.sync.dma_start(
                        hb_out[:].rearrange("(p jt) -> p jt", p=128),
                        hcat[:, KT:2 * KT])

                # ================= output projection =================
                with tc.tile_pool(name="outp", bufs=1) as xp:
                    wo = xp.tile([128, R, 65], BF16, tag="wo")
                    scro = xp.tile([128, R, 65], BF16, tag="scro")
                    so = xp.tile([128, R], F32, tag="so")
                    orep = xp.tile([128, R], F32, tag="orep")

                    nc.sync.dma_start(
                        wo[:].rearrange("p a b -> p (a b)"), wo_d[:])
                    hcb = hcat[:].rearrange(
                        "p (o t) -> p o t", o=1).broadcast_to([128, R, 65])
                    nc.vector.tensor_mul(scro[:], wo[:], hcb)
                    nc.vector.reduce_sum(so[:], scro[:], axis=AX.X)
                    nc.gpsimd.partition_all_reduce(orep[:], so[:], 128,
                                                   RED.add)
                    nc.sync.dma_start(
                        o_slice[:].rearrange("(o j) -> o j", o=1),
                        orep[0:1, :])

    nc.compile()
    return nc


_CACHE = {}


def _get_nc(niters=1):
    key = f"nc{niters}"
    if key not in _CACHE:
        _CACHE[key] = _build(niters)
    return _CACHE[key]


def _tile_rows(W, c, nt, bias=None, bias_div=128.0):
    """W rows [Rc, nt*32] -> [128, R, nt(+1)] bf16 (block-tiled contraction:
    slot t of partition p is contraction element nt*p+t, plus bias slot)."""
    sl = W[c * R:(c + 1) * R, :]
    r = sl.reshape(R, 128, nt).transpose(1, 0, 2)           # [128, R, nt]
    if bias is None:
        return np.ascontiguousarray(r).astype(BF)
    b = np.broadcast_to(bias[c * R:(c + 1) * R] / bias_div, (128, R))
    out = np.concatenate([r, b[:, :, None]], axis=2)
    return np.ascontiguousarray(out).astype(BF)


def _tile_vec(v):
    """[NH] -> [128, KT] block-tiled (element 32*p+jt)."""
    return np.ascontiguousarray(v.reshape(128, KT))


def kernel(x_f, x_b, h_f, h_b, t_f, t_b,
           i2h_W, i2h_b, h2o_W, h2o_b, f_W1, f_b1, f_W2, f_b2):
    x_f = np.asarray(x_f, np.float32).reshape(-1)
    x_b = np.asarray(x_b, np.float32).reshape(-1)
    h_f = np.asarray(h_f, np.float32)
    h_b = np.asarray(h_b, np.float32)
    t_f = np.asarray(t_f, np.float32)
    t_b = np.asarray(t_b, np.float32)
    i2h_W = np.asarray(i2h_W, np.float32)
    i2h_b = np.asarray(i2h_b, np.float32)
    h2o_W = np.asarray(h2o_W, np.float32)
    h2o_b = np.asarray(h2o_b, np.float32)
    f_W1 = np.asarray(f_W1, np.float32)
    f_b1 = np.asarray(f_b1, np.float32)
    f_W2 = np.asarray(f_W2, np.float32)
    f_b2 = np.asarray(f_b2, np.float32)

    nc = _get_nc(int(_CACHE.get("niters", 1)))

    # shared (core-independent) tensors
    xt = np.zeros((128, 2, 33), np.float32)
    xt[:, 0, :KT] = _tile_vec(x_f)
    xt[:, 1, :KT] = _tile_vec(x_b)
    xt[:, :, 32] = 1.0
    h0bf = np.zeros((128, 2, 33), np.float32)
    h0bf[:, 0, :KT] = _tile_vec(h_f)
    h0bf[:, 1, :KT] = _tile_vec(h_b)
    h0bf[:, :, 32] = 1.0
    h0f = np.stack([_tile_vec(h_f), _tile_vec(h_b)], axis=1)  # [128, 2, KT]

    cw = np.zeros((NSTEP, 4, 2, 2), np.float32)
    for ch, t in enumerate([t_f, t_b]):
        dt = (t[1:] - t[:-1]).astype(np.float32)
        cw[:, 0, 0, ch] = dt / 2.0
        cw[:, 1, 0, ch] = dt / 2.0
        cw[:, 2, 0, ch] = dt
        cw[:, 0, 1, ch] = dt / 6.0
        cw[:, 1, 1, ch] = dt / 3.0
        cw[:, 2, 1, ch] = dt / 3.0
        cw[:, 3, 1, ch] = dt / 6.0
    cw_b = np.ascontiguousarray(
        np.broadcast_to(cw.reshape(1, -1), (128, NSTEP * 16)), np.float32)

    xt_r = xt.reshape(128, 66).astype(BF)
    h0bf_r = h0bf.reshape(128, 66).astype(BF)
    h0f_r = np.ascontiguousarray(h0f.reshape(128, 64), np.float32)

    in_maps = []
    for c in range(NCORES):
        # w2f: [128, KT, R+1]: W2[32*p+jt, R*c+j], bias b2/8
        w2sl = f_W2[:, c * R:(c + 1) * R]                    # [NH, R]
        w2r = w2sl.reshape(128, KT, R)                       # [128, KT, R]
        b2s = (f_b2 / float(NCORES)).reshape(128, KT)
        w2f = np.concatenate([w2r, b2s[:, :, None]], axis=2).astype(BF)
        # wo: [128, R, 65]: cols (32p+tt) of h2o_x | (4096+32p+tt) of h2o_h
        wsl = h2o_W[c * R:(c + 1) * R]                       # [R, 2NH]
        wof = wsl[:, :NH].reshape(R, 128, KT).transpose(1, 0, 2)
        wob = wsl[:, NH:].reshape(R, 128, KT).transpose(1, 0, 2)
        bos = np.broadcast_to(h2o_b[c * R:(c + 1) * R] / 128.0, (128, R))
        wo = np.concatenate([wof, wob, bos[:, :, None]], axis=2).astype(BF)
        in_maps.append({
            "w1p": _tile_rows(f_W1, c, KT, f_b1).reshape(128, -1),
            "w2f": np.ascontiguousarray(w2f).reshape(128, -1),
            "wgx": _tile_rows(i2h_W[:, :NH], c, KT, i2h_b).reshape(128, -1),
            "wgh": _tile_rows(i2h_W[:, NH:], c, KT).reshape(128, -1),
            "wo": np.ascontiguousarray(wo).reshape(128, -1),
            "xt": xt_r, "h0bf": h0bf_r, "h0f": h0f_r, "cw": cw_b,
        })

    res = bass_utils.run_bass_kernel_spmd(nc, in_maps,
                                          core_ids=list(range(NCORES)))
    _CACHE["last_results"] = res

    out = np.concatenate(
        [np.asarray(res.results[c]["o_slice"], np.float32)
         for c in range(NCORES)])
    hf = np.asarray(res.results[0]["hf_out"]).astype(np.float32)
    hb = np.asarray(res.results[0]["hb_out"]).astype(np.float32)
    return out, hf, hb


# revision 29
# speedup vs baseline: 6.8342x; 6.8342x over previous
"""Trainium2 Bass kernel for the bidirectional GRU-ODE (nn_CODEBiGRU).

Strategy (8-way TP, latency-minimized: per-instruction overhead dominates on
this setup, so the design minimizes serial instruction count):
  - State h is partition-tiled [p, ch, jt] (element 128*jt+p), replicated on
    every core, f32.
  - f-eval per RK4 stage: u = W1@hstage+b1 via one DVE mul (per-partition
    partial products, bias folded as an extra contraction slot), one DVE
    reduce, one gpsimd partition_all_reduce (result replicated across
    partitions, so tanh output is directly mv2's contraction input).
    k-partial via W2 column-shard: one DVE mul + one reduce; partials summed
    across cores by a single 32KB ncfw AllReduce per stage.
  - RK4 coefficients folded into small const tensors; ~4 DVE update ops/stage.
  - GRU + h2o phase at the end reuses the same pattern (2 AllGathers).
"""
import sys
import numpy as np

sys.path.insert(0, "/opt/trn_rl_repo")

import ml_dtypes  # noqa: E402
import concourse.bass as bass  # noqa: E402
import concourse.tile as tile  # noqa: E402
import concourse.bass_isa as bass_isa  # noqa: E402
from concourse import bacc, mybir, bass_utils  # noqa: E402

NCORES = 8
NH = 4096
R = NH // NCORES        # 512 rows per core
KT = NH // 128          # 32 k-tiles
NSTEP = 15
F32 = mybir.dt.float32
BF16 = mybir.dt.bfloat16
AF = mybir.ActivationFunctionType
ALU = mybir.AluOpType
AX = mybir.AxisListType
RED = bass_isa.ReduceOp
GROUP = [list(range(NCORES))]
BF = ml_dtypes.bfloat16


def _bc1(ap, shape):
    """[p, a, b] -> [p, N, a, b] broadcast (insert axis after partition)."""
    a = ap.shape[1]
    return ap.rearrange("p (o a) b -> p o a b", o=1, a=a).broadcast_to(shape)


def _bc2(ap, shape):
    """[p, ch, x] -> [p, ch, N, x] broadcast (insert axis 2)."""
    x = ap.shape[2]
    return ap.rearrange("p ch (o x) -> p ch o x", o=1, x=x).broadcast_to(shape)


def _build(niters=1):
    nc = bacc.Bacc("TRN2", target_bir_lowering=False, debug=False,
                   num_devices=NCORES)

    # ---- kernel I/O (per-core prepped on host) ----
    w1p_d = nc.dram_tensor("w1p", [128, R * 33], BF16, kind="ExternalInput")
    w2f_d = nc.dram_tensor("w2f", [128, KT * (R + 1)], BF16, kind="ExternalInput")
    wgx_d = nc.dram_tensor("wgx", [128, R * 33], BF16, kind="ExternalInput")
    wgh_d = nc.dram_tensor("wgh", [128, R * 32], BF16, kind="ExternalInput")
    wo_d = nc.dram_tensor("wo", [128, R * 65], BF16, kind="ExternalInput")
    xt_d = nc.dram_tensor("xt", [128, 66], BF16, kind="ExternalInput")
    h0bf_d = nc.dram_tensor("h0bf", [128, 66], BF16, kind="ExternalInput")
    h0f_d = nc.dram_tensor("h0f", [128, 64], F32, kind="ExternalInput")
    cw_d = nc.dram_tensor("cw", [128, NSTEP * 16], F32, kind="ExternalInput")

    o_slice = nc.dram_tensor("o_slice", [R], F32, kind="ExternalOutput")
    hf_out = nc.dram_tensor("hf_out", [NH], BF16, kind="ExternalOutput")
    hb_out = nc.dram_tensor("hb_out", [NH], BF16, kind="ExternalOutput")

    with tile.TileContext(nc) as tc:
        with tc.tile_pool(name="persist", bufs=1) as pp, \
             tc.tile_pool(name="dram", bufs=2, space="DRAM") as dram:

            h_t = pp.tile([128, 2, KT], F32, tag="h_t")
            xt = pp.tile([128, 2, 33], BF16, tag="xt")
            cw = pp.tile([128, NSTEP, 4, 2, 2], F32, tag="cw")
            hcat = pp.tile([128, 65], BF16, tag="hcat")

            arin = dram.tile([128, 64], F32, tag="arin", name="arin")
            arout = dram.tile([128, 64], F32, tag="arout", name="arout")
            agin = dram.tile([1, 2 * R], F32, tag="agin", name="agin")
            agout = dram.tile([NCORES, 2 * R], F32, tag="agout", name="agout")
            ag2in = dram.tile([1, 2 * R], F32, tag="ag2in", name="ag2in")
            ag2out = dram.tile([NCORES, 2 * R], F32, tag="ag2out", name="ag2out")

            for _it in range(niters):
                nc.sync.dma_start(xt[:].rearrange("p a b -> p (a b)"), xt_d[:])
                nc.sync.dma_start(h_t[:].rearrange("p a b -> p (a b)"),
                                  h0f_d[:])
                nc.sync.dma_start(
                    cw[:].rearrange("p a b c d -> p (a b c d)"), cw_d[:])

                # ================= ODE phase =================
                with tc.tile_pool(name="ode", bufs=1) as op_:
                    w1p = op_.tile([128, R, 33], BF16, tag="w1p")
                    w2f = op_.tile([128, KT, R + 1], BF16, tag="w2f")
                    scr = op_.tile([128, 2 * R * 33], BF16, tag="scr")
                    s1 = op_.tile([128, 2, R], F32, tag="s1")
                    urep = op_.tile([128, 2, R], F32, tag="urep")
                    t_rep = op_.tile([128, 2, R + 1], BF16, tag="t_rep")
                    hst = op_.tile([128, 2, 33], BF16, tag="hst")
                    hacc = op_.tile([128, 2, KT], F32, tag="hacc")
                    kf = op_.tile([128, 2, KT], F32, tag="kf")
                    kpart = op_.tile([128, 2, KT], F32, tag="kpart")
                    tmpv = op_.tile([128, 2, KT], F32, tag="tmpv")
                    tmpw = op_.tile([128, 2, KT], F32, tag="tmpw")

                    nc.sync.dma_start(
                        w1p[:].rearrange("p a b -> p (a b)"), w1p_d[:])
                    nc.sync.dma_start(
                        w2f[:].rearrange("p a b -> p (a b)"), w2f_d[:])
                    nc.sync.dma_start(
                        hst[:].rearrange("p a b -> p (a b)"), h0bf_d[:])
                    nc.gpsimd.memset(t_rep[:, :, R:R + 1], 1.0)

                    A = scr[:].rearrange("p (ch j t) -> p ch j t", ch=2, j=R)
                    Bv = scr[:, :2 * KT * (R + 1)].rearrange(
                        "p (ch jt j) -> p ch jt j", ch=2, jt=KT)
                    w1b = _bc1(w1p[:], [128, 2, R, 33])
                    w2b = _bc1(w2f[:], [128, 2, KT, R + 1])

                    deferred = []
                    for s in range(NSTEP):
                        for q in range(4):
                            # u = W1 @ hstage + b1 (partials -> all partitions)
                            nc.vector.tensor_mul(
                                A, w1b, _bc2(hst[:], [128, 2, R, 33]))
                            nc.vector.reduce_sum(s1[:], A, axis=AX.X)
                            # off-critical-path bookkeeping from the previous
                            # stage runs while par/tanh own the chain
                            for fn in deferred:
                                fn()
                            deferred = []
                            nc.gpsimd.partition_all_reduce(
                                urep[:].rearrange("p a b -> p (a b)"),
                                s1[:].rearrange("p a b -> p (a b)"),
                                128, RED.add)
                            nc.scalar.activation(t_rep[:, :, 0:R], urep[:],
                                                 AF.Tanh)
                            # k_partial = W2c @ t + b2/8 (tiled layout)
                            nc.vector.tensor_mul(
                                Bv, w2b, _bc2(t_rep[:], [128, 2, KT, R + 1]))
                            nc.vector.reduce_sum(kpart[:], Bv, axis=AX.X)
                            nc.gpsimd.dma_start(
                                arin[:], kpart[:].rearrange("p a b -> p (a b)"))
                            nc.gpsimd.collective_compute(
                                "AllReduce", ALU.add, replica_groups=GROUP,
                                ins=[arin.opt()], outs=[arout.opt()])
                            nc.gpsimd.dma_start(
                                kf[:].rearrange("p a b -> p (a b)"), arout[:])
                            # RK4 bookkeeping (critical: hst only; h/hacc
                            # bookkeeping deferred into the next stage's
                            # par/tanh shadow)
                            wqb = cw[:, s, q, 1, :].rearrange(
                                "p (ch o) -> p ch o", o=1
                            ).broadcast_to([128, 2, KT])
                            if q < 3:
                                cqb = cw[:, s, q, 0, :].rearrange(
                                    "p (ch o) -> p ch o", o=1
                                ).broadcast_to([128, 2, KT])
                                nc.vector.tensor_mul(tmpv[:], kf[:], cqb)
                                nc.vector.tensor_add(hst[:, :, 0:KT], h_t[:],
                                                     tmpv[:])

                                def _acc(q=q, wqb=wqb):
                                    nc.vector.tensor_mul(tmpw[:], kf[:], wqb)
                                    nc.vector.tensor_add(
                                        hacc[:], h_t[:] if q == 0 else hacc[:],
                                        tmpw[:])
                                deferred.append(_acc)
                            else:
                                nc.vector.tensor_mul(tmpv[:], kf[:], wqb)
                                if s < NSTEP - 1:
                                    nc.vector.tensor_add(hst[:, :, 0:KT],
                                                         hacc[:], tmpv[:])

                                def _hnew():
                                    nc.vector.tensor_add(h_t[:], hacc[:],
                                                         tmpv[:])
                                deferred.append(_hnew)
                    for fn in deferred:
                        fn()
                    deferred = []

                # ================= GRU + output phase =================
                with tc.tile_pool(name="gru", bufs=1) as gp:
                    wgx = gp.tile([128, R, 33], BF16, tag="wgx")
                    wgh = gp.tile([128, R, 32], BF16, tag="wgh")
                    scrg = gp.tile([128, 2 * R * 33], BF16, tag="scrg")
                    sx = gp.tile([128, 2, R], F32, tag="sx")
                    sh = gp.tile([128, 2, R], F32, tag="sh")
                    ug = gp.tile([128, 2, R], F32, tag="ug")
                    gs = gp.tile([128, R, 2], F32, tag="gs")
                    hh1 = gp.tile([1, R, 2], F32, tag="hh1")
                    vh = gp.tile([128, 2, KT], BF16, tag="vh")
                    gful = gp.tile([128, KT, 2], F32, tag="gful")
                    hhful = gp.tile([128, KT, 2], F32, tag="hhful")
                    tmpg = gp.tile([128, 2, KT], F32, tag="tmpg")

                    nc.sync.dma_start(
                        wgx[:].rearrange("p a b -> p (a b)"), wgx_d[:])
                    nc.sync.dma_start(
                        wgh[:].rearrange("p a b -> p (a b)"), wgh_d[:])

                    Ax = scrg[:].rearrange("p (ch j t) -> p ch j t", ch=2, j=R)
                    Ah = scrg[:, :2 * R * 32].rearrange(
                        "p (ch j t) -> p ch j t", ch=2, j=R)
                    wgxb = _bc1(wgx[:], [128, 2, R, 33])
                    wghb = _bc1(wgh[:], [128, 2, R, 32])

                    # ux = i2h_x @ x + bg  (partial, reused by both matvecs)
                    nc.vector.tensor_mul(Ax, wgxb, _bc2(xt[:], [128, 2, R, 33]))
                    nc.vector.reduce_sum(sx[:], Ax, axis=AX.X)
                    # uh = i2h_h @ h
                    nc.vector.tensor_copy(vh[:], h_t[:])
                    nc.vector.tensor_mul(Ah, wghb, _bc2(vh[:], [128, 2, R, 32]))
                    nc.vector.reduce_sum(sh[:], Ah, axis=AX.X)
                    nc.vector.tensor_add(sh[:], sx[:], sh[:])
                    nc.gpsimd.partition_all_reduce(
                        ug[:].rearrange("p a b -> p (a b)"),
                        sh[:].rearrange("p a b -> p (a b)"), 128, RED.add)
                    nc.scalar.activation(
                        gs[:].rearrange("p r ch -> p ch r"), ug[:],
                        AF.Sigmoid)
                    # AllGather g (block [r, ch]: flat idx 64p+2jt+ch)
                    nc.sync.dma_start(
                        agin[:], gs[0:1, :, :].rearrange("o r ch -> o (r ch)"))
                    nc.gpsimd.collective_compute(
                        "AllGather", ALU.bypass, replica_groups=GROUP,
                        ins=[agin.opt()], outs=[agout.opt()])
                    nc.sync.dma_start(
                        gful[:],
                        agout[:].rearrange("a rc -> (a rc)").rearrange(
                            "(p jt ch) -> p jt ch", p=128, ch=2))
                    # h_hat = tanh(ux + i2h_h @ (g*h))
                    gfv = gful[:].rearrange("p jt ch -> p ch jt")
                    hhv = hhful[:].rearrange("p jt ch -> p ch jt")
                    nc.vector.tensor_mul(vh[:], gfv, h_t[:])
                    nc.vector.tensor_mul(Ah, wghb, _bc2(vh[:], [128, 2, R, 32]))
                    nc.vector.reduce_sum(sh[:], Ah, axis=AX.X)
                    nc.vector.tensor_add(sh[:], sx[:], sh[:])
                    nc.gpsimd.partition_all_reduce(
                        ug[:].rearrange("p a b -> p (a b)"),
                        sh[:].rearrange("p a b -> p (a b)"), 128, RED.add)
                    nc.scalar.activation(
                        hh1[:].rearrange("o r ch -> o ch r"),
                        ug[0:1, :, :], AF.Tanh)
                    # AllGather h_hat
                    nc.sync.dma_start(
                        ag2in[:], hh1[:].rearrange("o r ch -> o (r ch)"))
                    nc.gpsimd.collective_compute(
                        "AllGather", ALU.bypass, replica_groups=GROUP,
                        ins=[ag2in.opt()], outs=[ag2out.opt()])
                    nc.sync.dma_start(
                        hhful[:],
                        ag2out[:].rearrange("a rc -> (a rc)").rearrange(
                            "(p jt ch) -> p jt ch", p=128, ch=2))
                    # h_new = hh + g*(h - hh)  -> hcat (bf16)
                    nc.vector.tensor_sub(tmpg[:], h_t[:], hhv)
                    nc.vector.tensor_mul(tmpg[:], gfv, tmpg[:])
                    nc.gpsimd.memset(hcat[:, 64:65], 1.0)
                    nc.vector.tensor_add(
                        hcat[:, 0:64].rearrange("p (ch jt) -> p ch jt", ch=2),
                        hhv, tmpg[:])
                    nc.sync.dma_start(
                        hf_out[:].rearrange("(p jt) -> p jt", p=128),
                        hcat[:, 0:KT])
                    nc.sync.dma_start(
                        hb_out[:].rearrange("(p jt) -> p jt", p=128),
                        hcat[:, KT:2 * KT])

                # ================= output projection =================
                with tc.tile_pool(name="outp", bufs=1) as xp:
                    wo = xp.tile([128, R, 65], BF16, tag="wo")
                    scro = xp.tile([128, R, 65], BF16, tag="scro")
                    so = xp.tile([128, R], F32, tag="so")
                    orep = xp.tile([128, R], F32, tag="orep")

                    nc.sync.dma_start(
                        wo[:].rearrange("p a b -> p (a b)"), wo_d[:])
                    hcb = hcat[:].rearrange(
                        "p (o t) -> p o t", o=1).broadcast_to([128, R, 65])
                    nc.vector.tensor_mul(scro[:], wo[:], hcb)
                    nc.vector.reduce_sum(so[:], scro[:], axis=AX.X)
                    nc.gpsimd.partition_all_reduce(orep[:], so[:], 128,
                                                   RED.add)
                    nc.sync.dma_start(
                        o_slice[:].rearrange("(o j) -> o j", o=1),
                        orep[0:1, :])

    nc.compile()
    return nc


_CACHE = {}


def _get_nc(niters=1):
    key = f"nc{niters}"
    if key not in _CACHE:
        _CACHE[key] = _build(niters)
    return _CACHE[key]


def _tile_rows(W, c, nt, bias=None, bias_div=128.0):
    """W rows [Rc, nt*32] -> [128, R, nt(+1)] bf16 (block-tiled contraction:
    slot t of partition p is contraction element nt*p+t, plus bias slot)."""
    sl = W[c * R:(c + 1) * R, :]
    r = sl.reshape(R, 128, nt).transpose(1, 0, 2)           # [128, R, nt]
    if bias is None:
        return np.ascontiguousarray(r).astype(BF)
    b = np.broadcast_to(bias[c * R:(c + 1) * R] / bias_div, (128, R))
    out = np.concatenate([r, b[:, :, None]], axis=2)
    return np.ascontiguousarray(out).astype(BF)


def _tile_vec(v):
    """[NH] -> [128, KT] block-tiled (element 32*p+jt)."""
    return np.ascontiguousarray(v.reshape(128, KT))


def kernel(x_f, x_b, h_f, h_b, t_f, t_b,
           i2h_W, i2h_b, h2o_W, h2o_b, f_W1, f_b1, f_W2, f_b2):
    x_f = np.asarray(x_f, np.float32).reshape(-1)
    x_b = np.asarray(x_b, np.float32).reshape(-1)
    h_f = np.asarray(h_f, np.float32)
    h_b = np.asarray(h_b, np.float32)
    t_f = np.asarray(t_f, np.float32)
    t_b = np.asarray(t_b, np.float32)
    i2h_W = np.asarray(i2h_W, np.float32)
    i2h_b = np.asarray(i2h_b, np.float32)
    h2o_W = np.asarray(h2o_W, np.float32)
    h2o_b = np.asarray(h2o_b, np.float32)
    f_W1 = np.asarray(f_W1, np.float32)
    f_b1 = np.asarray(f_b1, np.float32)
    f_W2 = np.asarray(f_W2, np.float32)
    f_b2 = np.asarray(f_b2, np.float32)

    nc = _get_nc(int(_CACHE.get("niters", 1)))

    # shared (core-independent) tensors
    xt = np.zeros((128, 2, 33), np.float32)
    xt[:, 0, :KT] = _tile_vec(x_f)
    xt[:, 1, :KT] = _tile_vec(x_b)
    xt[:, :, 32] = 1.0
    h0bf = np.zeros((128, 2, 33), np.float32)
    h0bf[:, 0, :KT] = _tile_vec(h_f)
    h0bf[:, 1, :KT] = _tile_vec(h_b)
    h0bf[:, :, 32] = 1.0
    h0f = np.stack([_tile_vec(h_f), _tile_vec(h_b)], axis=1)  # [128, 2, KT]

    cw = np.zeros((NSTEP, 4, 2, 2), np.float32)
    for ch, t in enumerate([t_f, t_b]):
        dt = (t[1:] - t[:-1]).astype(np.float32)
        cw[:, 0, 0, ch] = dt / 2.0
        cw[:, 1, 0, ch] = dt / 2.0
        cw[:, 2, 0, ch] = dt
        cw[:, 0, 1, ch] = dt / 6.0
        cw[:, 1, 1, ch] = dt / 3.0
        cw[:, 2, 1, ch] = dt / 3.0
        cw[:, 3, 1, ch] = dt / 6.0
    cw_b = np.ascontiguousarray(
        np.broadcast_to(cw.reshape(1, -1), (128, NSTEP * 16)), np.float32)

    xt_r = xt.reshape(128, 66).astype(BF)
    h0bf_r = h0bf.reshape(128, 66).astype(BF)
    h0f_r = np.ascontiguousarray(h0f.reshape(128, 64), np.float32)

    in_maps = []
    for c in range(NCORES):
        # w2f: [128, KT, R+1]: W2[32*p+jt, R*c+j], bias b2/8
        w2sl = f_W2[:, c * R:(c + 1) * R]                    # [NH, R]
        w2r = w2sl.reshape(128, KT, R)                       # [128, KT, R]
        b2s = (f_b2 / float(NCORES)).reshape(128, KT)
        w2f = np.concatenate([w2r, b2s[:, :, None]], axis=2).astype(BF)
        # wo: [128, R, 65]: cols (32p+tt) of h2o_x | (4096+32p+tt) of h2o_h
        wsl = h2o_W[c * R:(c + 1) * R]                       # [R, 2NH]
        wof = wsl[:, :NH].reshape(R, 128, KT).transpose(1, 0, 2)
        wob = wsl[:, NH:].reshape(R, 128, KT).transpose(1, 0, 2)
        bos = np.broadcast_to(h2o_b[c * R:(c + 1) * R] / 128.0, (128, R))
        wo = np.concatenate([wof, wob, bos[:, :, None]], axis=2).astype(BF)
        in_maps.append({
            "w1p": _tile_rows(f_W1, c, KT, f_b1).reshape(128, -1),
            "w2f": np.ascontiguousarray(w2f).reshape(128, -1),
            "wgx": _tile_rows(i2h_W[:, :NH], c, KT, i2h_b).reshape(128, -1),
            "wgh": _tile_rows(i2h_W[:, NH:], c, KT).reshape(128, -1),
            "wo": np.ascontiguousarray(wo).reshape(128, -1),
            "xt": xt_r, "h0bf": h0bf_r, "h0f": h0f_r, "cw": cw_b,
        })

    res = bass_utils.run_bass_kernel_spmd(nc, in_maps,
                                          core_ids=list(range(NCORES)))
    _CACHE["last_results"] = res

    out = np.concatenate(
        [np.asarray(res.results[c]["o_slice"], np.float32)
         for c in range(NCORES)])
    hf = np.asarray(res.results[0]["hf_out"]).astype(np.float32)
    hb = np.asarray(res.results[0]["hb_out"]).astype(np.float32)
    return out, hf, hb
